# revision 37
# baseline (speedup 1.0000x reference)
"""Trainium2 Bass kernel for nn_CrossAttention (16x512x64x64, 8 heads x 64).

Math notes (exact algebraic restructuring of the reference):
  The reference tiles ky=[b,1,1,c] to k=[b,c,1,c] before conv1x1(to_k_w), so
  every input channel of that conv carries the same value ky[b,j].  Hence
    conv1x1(k, to_k_w)[b,o,0,j] = rowsum(to_k_w)[o] * ky[b,j]     (rank-1)
  and likewise for v with rowsum(to_v_w) and vy.  Propagating this:
    ksm[b,hd,j] = softmax_j(rs_k[hd] * ky[b,j])
    w[b,hd]     = sum_j ksm[b,hd,j] * vy[b,j]
    s[b,h,n]    = num/den,  num = sum_d w[hd] e^{q[hd,n]}, den = sum_d e^{q[hd,n]}
    mm[b,o,n]   = sum_h W2[o,h] * s[b,h,n] + out_b[o],
      with W2[o,h] = scale * sum_e out_w[o, h*64+e] * rs_v[h*64+e]
  followed by GroupNorm(1) over (C,H,W) per sample:
    out = A[o]*mmW2[o,n] + B[o],  A = gn_g*rstd, B = A*(out_b-mu)+gn_b
  GN stats come from the 9x9 Gram matrix of [s; 1] over n:
    sum mm   = sum_ab Cm[a,b] S2[a,b],   sum mm^2 = sum_ab Gm[a,b] S2[a,b]
  where S2 = [s;1][s;1]^T (accumulated on PE), Cm/Gm folded on host from
  W2 / out_b.

Device layout: q kept transposed [he, n] so the d-softmax reductions are
small PE matmuls (block-diagonal masks carrying w), not DVE reductions.
The only large compute is the q GEMM (to_q_w @ x, bf16, 2.1 GFLOP/sample).

Sharding: data-parallel over batch, 2 samples per core, 8 cores, no
collectives.  Weight folding (transposes, rowsums, W2, Gm/Cm) is done on
host; x is staged to bf16 on host.
"""

import numpy as np
import ml_dtypes

import concourse.bass as bass
import concourse.mybir as mybir
import concourse.tile as tile
from concourse import bacc
from concourse.bass import ts
from concourse.bass_utils import run_bass_kernel_spmd

B, C, N = 16, 512, 4096
DIMY = 768
HEADS, DHEAD = 8, 64
NCORES = 8
BPC = B // NCORES  # samples per core
SCALE = DHEAD ** -0.5
EPS = 1e-5
F32 = mybir.dt.float32
BF16 = mybir.dt.bfloat16
AX = mybir.AxisListType.X
AF = mybir.ActivationFunctionType
OP = mybir.AluOpType
NG = 8          # n-groups of 512 per sample
GSZ = N // NG   # 512
CN = C * N

BF = ml_dtypes.bfloat16


def build_nc():
    nc = bacc.Bacc()
    xd = nc.dram_tensor("x", [BPC, 128, 4, N], BF16, kind="ExternalInput")
    qwTd = nc.dram_tensor("qwT", [128, 4, C], BF16, kind="ExternalInput")
    kvTd = nc.dram_tensor("kvT", [128, 6, 2 * C], BF16, kind="ExternalInput")
    yTd = nc.dram_tensor("yT", [128, 6, BPC], BF16, kind="ExternalInput")
    rskbd = nc.dram_tensor("rskb", [128, C], F32, kind="ExternalInput")
    omaskd = nc.dram_tensor("omask", [128, 4, HEADS], BF16, kind="ExternalInput")
    w2Td = nc.dram_tensor("w2T", [HEADS, 4, 128], BF16, kind="ExternalInput")
    gcmd = nc.dram_tensor("gcm", [9, 2, 9], F32, kind="ExternalInput")
    colsd = nc.dram_tensor("cols", [128, 12], F32, kind="ExternalInput")
    outd = nc.dram_tensor("out", [BPC, 4, 128, N], BF16, kind="ExternalOutput")

    from contextlib import ExitStack

    with tile.TileContext(nc) as tc, ExitStack() as ctx:
        persist = ctx.enter_context(tc.tile_pool(name="persist", bufs=1))
        xp = ctx.enter_context(tc.tile_pool(name="xp", bufs=2))
        ep = ctx.enter_context(tc.tile_pool(name="ep", bufs=3))
        stgp = ctx.enter_context(tc.tile_pool(name="stgp", bufs=4))
        samp = ctx.enter_context(tc.tile_pool(name="samp", bufs=2))
        gp = ctx.enter_context(tc.tile_pool(name="gp", bufs=3))
        tiny = ctx.enter_context(tc.tile_pool(name="tiny", bufs=4))
        psqp = ctx.enter_context(tc.tile_pool(name="psqp", bufs=2, space="PSUM"))
        psndp = ctx.enter_context(tc.tile_pool(name="psndp", bufs=2, space="PSUM"))
        psf2p = ctx.enter_context(tc.tile_pool(name="psf2p", bufs=2, space="PSUM"))
        psgp = ctx.enter_context(tc.tile_pool(name="psgp", bufs=1, space="PSUM"))
        pssm = ctx.enter_context(tc.tile_pool(name="pssm", bufs=1, space="PSUM"))

        # ---------------- prep: weights + constants ----------------
        # The ky matmuls head the PE queue, so their inputs (yT + kvT) go
        # first, kvT in per-dt chunks so the first matmul releases early.
        yT = persist.tile([128, 6, BPC], BF16, tag="yT")
        nc.sync.dma_start(out=yT, in_=yTd[:, :, :])
        kvT = persist.tile([128, 6, 2 * C], BF16, tag="kvT")
        for dt_ in range(6):
            nc.sync.dma_start(out=kvT[:, dt_, :], in_=kvTd[:, dt_, :])
        qwT = persist.tile([128, 4, C], BF16, tag="qwT")
        nc.sync.dma_start(out=qwT, in_=qwTd[:, :, :])
        rskb = persist.tile([128, C], F32, tag="rskb")
        nc.sync.dma_start(out=rskb, in_=rskbd[:, :])
        omask = persist.tile([128, 4, HEADS], BF16, tag="omask")
        nc.sync.dma_start(out=omask, in_=omaskd[:, :, :])
        w2T = persist.tile([HEADS, 4, 128], BF16, tag="w2T")
        nc.sync.dma_start(out=w2T, in_=w2Td[:, :, :])
        gcm = persist.tile([9, 2, 9], F32, tag="gcm")
        nc.sync.dma_start(out=gcm, in_=gcmd[:, :, :])
        gcols = persist.tile([128, 12], F32, tag="gcols")
        nc.sync.dma_start(out=gcols, in_=colsd[:, :])

        ident = persist.tile([128, 128], F32, tag="ident")
        from concourse.masks import make_identity

        make_identity(nc, ident)
        identb = persist.tile([128, 128], BF16, tag="identb")
        make_identity(nc, identb)
        ones_row = persist.tile([1, 128], F32, tag="onesr")
        nc.vector.memset(ones_row, 1.0)
        ones9 = persist.tile([9, 1], F32, tag="ones9")
        nc.vector.memset(ones9, 1.0)
        zero_col = persist.tile([128, 1], F32, tag="zero")
        nc.vector.memset(zero_col, 0.0)
        nc.const_aps.aps[(F32, 0.0)] = zero_col[:, :]
        eps_col = persist.tile([128, 1], F32, tag="eps")
        nc.vector.memset(eps_col, EPS)
        nc.const_aps.aps[(F32, EPS)] = eps_col[:, :]
        # gram staging: [128 n, 4 j, 9]; col 8 of each j-block stays 1.0
        gstage = persist.tile([128, 4, 9], BF16, tag="gstage")
        nc.vector.memset(gstage[:, :, 8:9], 1.0)
        # s values for both samples: [8 h, s, n]
        s_all = persist.tile([HEADS, BPC, N], BF16, tag="s_all")

        # x: sample 0 split across three queues so the first q matmuls
        # release early; sample 1 on the sync queue behind the weights
        xts = []
        x0 = xp.tile([128, 4, N], BF16, tag="xt", name="x0")
        H0 = N // 2
        for ct, eng in ((0, nc.scalar), (1, nc.scalar), (2, nc.gpsimd), (3, nc.sync)):
            eng.dma_start(out=x0[:, ct, 0:H0], in_=xd[0][:, ct, 0:H0])
        for ct, eng in ((0, nc.scalar), (1, nc.scalar), (2, nc.gpsimd), (3, nc.sync)):
            eng.dma_start(out=x0[:, ct, H0:N], in_=xd[0][:, ct, H0:N])
        x1 = xp.tile([128, 4, N], BF16, tag="xt", name="x1")
        nc.sync.dma_start(out=x1, in_=xd[1])
        xts = [x0, x1]

        # ---------------- ky/vy for both samples ----------------
        # ky[s, o] = sum_d y[s, d] k_w[o, d]; vy likewise (both via PE)
        krows = tiny.tile([BPC, 2, C], F32, tag="krows")
        for kv in range(2):
            ps_ky = pssm.tile([BPC, C], F32, tag="sm")
            for dt_ in range(6):
                nc.tensor.matmul(
                    ps_ky, lhsT=yT[:, dt_, :], rhs=kvT[:, dt_, kv * C : (kv + 1) * C],
                    start=(dt_ == 0), stop=(dt_ == 5),
                )
            nc.vector.tensor_copy(out=krows[:, kv, :], in_=ps_ky)
        # transpose to columns: kv_cols[p, 4*ot + 2*kv + s]
        ps_kc = pssm.tile([128, 16], F32, tag="sm")
        for ot in range(4):
            for kv in range(2):
                nc.tensor.transpose(
                    ps_kc[:, 4 * ot + 2 * kv : 4 * ot + 2 * kv + 2],
                    krows[:, kv, ts(ot, 128)],
                    ident[0:BPC, 0:BPC],
                )
        kv_cols = persist.tile([128, 16], F32, tag="kvcols")
        nc.vector.tensor_copy(out=kv_cols, in_=ps_kc)

        # ---------------- k-softmax -> w, masks ----------------
        numqs = {}

        def kv_path(s):
            # E_T[j, hd] = exp(ky[j] * rs_k[hd])
            et = ep.tile([128, 4, C], BF16, tag="eq")
            for jt in range(4):
                nc.scalar.activation(
                    out=et[:, jt, :], in_=rskb, func=AF.Exp,
                    scale=kv_cols[:, 4 * jt + s : 4 * jt + s + 1],
                )
            # masks: col 0 = vy (num), col 32 = 1 (den at psum partition 32)
            kvm = tiny.tile([128, 4, 33], BF16, tag="kvm")
            nc.vector.memset(kvm, 0.0)
            nc.vector.tensor_copy(
                out=kvm[:, :, 0:1],
                in_=kv_cols.rearrange("p (a r) -> p a r", r=4)[:, :, 2 + s : 3 + s],
            )
            nc.vector.memset(kvm[:, :, 32:33], 1.0)
            ps_w = pssm.tile([33, C], F32, tag="sm")
            for jt in range(4):
                nc.tensor.matmul(
                    ps_w, lhsT=kvm[:, jt, :], rhs=et[:, jt, :],
                    start=(jt == 0), stop=(jt == 3),
                )
            dwsb = tiny.tile([1, 2, C], F32, tag="dwsb")
            nc.vector.tensor_copy(out=dwsb[:, 0, :], in_=ps_w[32:33, :])
            nc.vector.reciprocal_approx_fast(out=dwsb[:, 1, :], in_=dwsb[:, 0, :])
            w_row = tiny.tile([1, C], F32, tag="wrow")
            nc.vector.tensor_mul(w_row, ps_w[0:1, :], dwsb[:, 1, :])
            ps_wc = pssm.tile([128, 4], F32, tag="sm")
            for ht in range(4):
                nc.tensor.transpose(
                    ps_wc[:, ht : ht + 1], w_row[:, ts(ht, 128)], ident[0:1, 0:1]
                )
            w_col = tiny.tile([128, 4], F32, tag="wcol")
            nc.vector.tensor_copy(out=w_col, in_=ps_wc)
            # numq masks: cols 0:8 = omask * w (num), cols 32:40 = omask (den)
            numq = samp.tile([128, 4, 48], BF16, tag="numq")
            nc.vector.memset(numq, 0.0)
            for ht in range(4):
                nc.vector.tensor_scalar(
                    out=numq[:, ht, 0:HEADS], in0=omask[:, ht, :],
                    scalar1=w_col[:, ht : ht + 1], scalar2=None, op0=OP.mult,
                )
            nc.vector.tensor_copy(out=numq[:, :, 32:40], in_=omask)
            numqs[s] = numq

        # ---------------- per-sample pass 1 ----------------
        psgs = {}

        def emit_gram(s, g):
            ps_sT = pssm.tile([128, 4 * HEADS], BF16, tag="sm")
            for j in range(4):
                nc.tensor.transpose(
                    ps_sT[:, 8 * j : 8 * j + 8],
                    s_all[:, s, g * GSZ + 128 * j : g * GSZ + 128 * (j + 1)],
                    identb[0:HEADS, 0:HEADS],
                )
            nc.vector.tensor_copy(
                out=gstage[:, :, 0:HEADS],
                in_=ps_sT.rearrange("p (j h) -> p j h", h=HEADS),
            )
            for j in range(4):
                nc.tensor.matmul(
                    psgs[s][:, j, :], lhsT=gstage[:, j, :], rhs=gstage[:, j, :],
                    start=(g == 0), stop=(g == NG - 1), skip_group_check=True,
                )

        def pass1_group(s, g, psnd_box, extra=None):
            """One n-group of 512: q GEMM, exp, nd matmuls; every odd group
            finishes the pair (reciprocal + muls).  Gram work for groups g-3,
            g-2 is emitted first so the PE never waits on the DVE chain.
            `extra` emits interleaved work (pass2 units of the other sample)."""
            if g >= 3 and g % 2 == 1:
                emit_gram(s, g - 3)
                emit_gram(s, g - 2)
            eq = ep.tile([128, 4, C], BF16, tag="eq")
            for ht in range(4):
                psq = psqp.tile([128, GSZ], F32, tag="psq")
                for ct in range(4):
                    nc.tensor.matmul(
                        psq,
                        lhsT=qwT[:, ct, ts(ht, 128)],
                        rhs=xts[s][:, ct, ts(g, GSZ)],
                        start=(ct == 0), stop=(ct == 3),
                    )
                nc.scalar.activation(out=eq[:, ht, :], in_=psq, func=AF.Exp)
            if g % 2 == 0:
                psnd_box[0] = psndp.tile([112, GSZ], F32, tag="nd", name="psnd")
            psnd = psnd_box[0]
            base = 64 * (g % 2)
            for ht in range(4):
                nc.tensor.matmul(
                    psnd[base : base + 48, :],
                    lhsT=numqs[s][:, ht, :], rhs=eq[:, ht, :],
                    start=(ht == 0), stop=(ht == 3),
                )
            if extra is not None:
                extra()
            if g % 2 == 1:
                dsb = gp.tile([112, GSZ], F32, tag="dsb")
                nc.vector.tensor_copy(out=dsb, in_=psnd)
                rdf = gp.tile([112, GSZ], F32, tag="rden")
                nc.vector.reciprocal_approx_fast(out=rdf, in_=dsb)
                nc.vector.tensor_mul(
                    s_all[:, s, ts(g - 1, GSZ)], psnd[0:HEADS, :], rdf[32:40, :]
                )
                nc.vector.tensor_mul(
                    s_all[:, s, ts(g, GSZ)], psnd[64 : 64 + HEADS, :],
                    rdf[96:104, :],
                )

        def stats_a(s):
            """GN stats part A: Gram psum -> wred (pure DVE chain)."""
            psg = psgs[s]
            gsb = tiny.tile([9, 4, 9], F32, tag="gsb")
            nc.vector.tensor_copy(out=gsb, in_=psg)
            s2 = tiny.tile([9, 9], F32, tag="s2")
            nc.vector.reduce_sum(
                out=s2, in_=gsb.rearrange("p j b -> p b j"), axis=AX
            )
            work = tiny.tile([9, 2, 9], F32, tag="work")
            nc.vector.tensor_mul(work[:, 0, :], gcm[:, 0, :], s2)
            nc.vector.tensor_mul(work[:, 1, :], gcm[:, 1, :], s2)
            wred = tiny.tile([9, 2], F32, tag="wred")
            nc.vector.reduce_sum(out=wred, in_=work, axis=AX)
            return wred

        def stats_b(s, wred):
            """Part B: cross-partition sum, then bit-trick rsqrt (DVE)."""
            ps_s = pssm.tile([1, 2], F32, tag="sm")
            nc.tensor.matmul(ps_s, lhsT=ones9, rhs=wred, start=True, stop=True)
            msc = tiny.tile([1, 4], F32, tag="msc")
            nc.vector.tensor_scalar(
                out=msc[:, 0:2], in0=ps_s, scalar1=1.0 / CN, scalar2=None,
                op0=OP.mult,
            )
            nc.vector.tensor_mul(msc[:, 2:3], msc[:, 0:1], msc[:, 0:1])
            nc.vector.tensor_sub(msc[:, 3:4], msc[:, 1:2], msc[:, 2:3])
            # rstd = rsqrt(var + eps): quake seed + 3 Newton steps, all DVE
            nt = tiny.tile([1, 12], F32, tag="nt")
            nc.vector.tensor_scalar(
                out=nt[:, 0:1], in0=msc[:, 3:4], scalar1=EPS, scalar2=None,
                op0=OP.add,
            )
            v = nt[:, 0:1]
            nt_i = nt.bitcast(mybir.dt.int32)
            nc.vector.tensor_scalar(
                out=nt_i[:, 1:2], in0=nt_i[:, 0:1], scalar1=1, scalar2=None,
                op0=OP.arith_shift_right,
            )
            nc.vector.tensor_scalar(
                out=nt_i[:, 2:3], in0=nt_i[:, 1:2], scalar1=-1,
                scalar2=0x5F3759DF, op0=OP.mult, op1=OP.add,
            )
            y = nt[:, 2:3]
            for it in range(3):
                b0 = 3 + 3 * it
                nc.vector.tensor_mul(nt[:, b0 : b0 + 1], y, y)
                nc.vector.tensor_mul(nt[:, b0 + 1 : b0 + 2], nt[:, b0 : b0 + 1], v)
                nc.vector.tensor_scalar(
                    out=nt[:, b0 + 2 : b0 + 3], in0=nt[:, b0 + 1 : b0 + 2],
                    scalar1=-0.5, scalar2=1.5, op0=OP.mult, op1=OP.add,
                )
                ynew = tiny.tile([1, 1], F32, tag=f"yn{it}")
                nc.vector.tensor_mul(ynew, y, nt[:, b0 + 2 : b0 + 3])
                y = ynew
            murow = tiny.tile([1, 2], F32, tag="murow")
            nc.vector.tensor_copy(out=murow[:, 0:1], in_=msc[:, 0:1])
            nc.vector.tensor_copy(out=murow[:, 1:2], in_=y)
            return murow

        def stats_c(s, murow):
            """Part C: broadcast mu/rstd, build the affine columns."""
            ps_b = pssm.tile([128, 2], F32, tag="sm")
            nc.tensor.matmul(ps_b, lhsT=ones_row, rhs=murow, start=True, stop=True)
            msb = tiny.tile([128, 2], F32, tag="msb")
            nc.vector.tensor_copy(out=msb, in_=ps_b)
            # A = gn_g * rstd ; B = A*(out_b - mu) + gn_b
            ab = samp.tile([128, 2, 4], F32, tag="ab")
            nc.vector.tensor_scalar(
                out=ab[:, 0, :], in0=gcols[:, 0:4],
                scalar1=msb[:, 1:2], scalar2=None, op0=OP.mult,
            )
            t1 = tiny.tile([128, 2, 4], F32, tag="t1")
            nc.vector.tensor_scalar(
                out=t1[:, 0, :], in0=gcols[:, 8:12],
                scalar1=msb[:, 0:1], scalar2=None, op0=OP.subtract,
            )
            nc.vector.tensor_mul(t1[:, 1, :], ab[:, 0, :], t1[:, 0, :])
            nc.vector.tensor_add(ab[:, 1, :], t1[:, 1, :], gcols[:, 4:8])
            return ab

        def stats(s):
            return stats_c(s, stats_b(s, stats_a(s)))

        def pass2_pair(s, g0, ot, idx, ab, stg_eng=None):
            """Two adjacent n-groups (same ot) -> one [128, 1024] staging
            tile and a single out-DMA, halving sync-queue trigger cost."""
            stg2 = stgp.tile([128, 2, GSZ], BF16, tag="stg2")
            for k in range(2):
                if (idx + k) % 3 == 1:
                    psf = psqp.tile([128, GSZ], F32, tag="psq", name="psf_q")
                elif (idx + k) % 3 == 2:
                    psf = psndp.tile([128, GSZ], F32, tag="nd", name="psf_n")
                else:
                    psf = psf2p.tile([128, GSZ], F32, tag="psf")
                nc.tensor.matmul(
                    psf, lhsT=w2T[:, ot, :], rhs=s_all[:, s, ts(g0 + k, GSZ)],
                    start=True, stop=True,
                )
                # one DVE + one ACT copy per pair: the two run concurrently
                # so the pair's DMA releases after max() not sum()
                use_dve = k == (idx // 2) % 2
                if stg_eng is not None:
                    use_dve = stg_eng == "vector"
                if use_dve:
                    nc.vector.tensor_scalar(
                        out=stg2[:, k, :], in0=psf,
                        scalar1=ab[:, 0, ot : ot + 1],
                        scalar2=ab[:, 1, ot : ot + 1],
                        op0=OP.mult, op1=OP.add,
                    )
                else:
                    nc.scalar.activation(
                        out=stg2[:, k, :], in_=psf, func=AF.Identity,
                        scale=ab[:, 0, ot : ot + 1], bias=ab[:, 1, ot : ot + 1],
                    )
            nc.sync.dma_start(
                out=outd[s, ot, :, g0 * GSZ : (g0 + 2) * GSZ], in_=stg2
            )

        def pass2_unit(s, g, ot, idx, ab, tail=False, stg_eng=None):
            if tail and idx % 3 == 1:
                psf = psqp.tile([128, GSZ], F32, tag="psq", name="psf_q")
            elif tail and idx % 3 == 2:
                psf = psndp.tile([128, GSZ], F32, tag="nd", name="psf_n")
            else:
                psf = psf2p.tile([128, GSZ], F32, tag="psf")
            nc.tensor.matmul(
                psf, lhsT=w2T[:, ot, :], rhs=s_all[:, s, ts(g, GSZ)],
                start=True, stop=True,
            )
            stg = stgp.tile([128, GSZ], BF16, tag="stg")
            use_dve = (idx % 4 < 3) if tail else (idx % 3 < 2)
            if stg_eng is not None:
                use_dve = stg_eng == "vector"
            if use_dve:
                nc.vector.tensor_scalar(
                    out=stg, in0=psf,
                    scalar1=ab[:, 0, ot : ot + 1], scalar2=ab[:, 1, ot : ot + 1],
                    op0=OP.mult, op1=OP.add,
                )
            else:
                nc.scalar.activation(
                    out=stg, in_=psf, func=AF.Identity,
                    scale=ab[:, 0, ot : ot + 1], bias=ab[:, 1, ot : ot + 1],
                )
            nc.sync.dma_start(out=outd[s, ot, :, ts(g, GSZ)], in_=stg)

        # pass 1 of sample 0 (sample 1's kv chain emitted at group 1 so it
        # overlaps instead of delaying the first q matmuls)
        kv_path(0)
        psgs[0] = psgp.tile([9, 4, 9], F32, tag="gram", name="psg0")
        box = [None]
        for g in range(NG):
            extra = (lambda: kv_path(1)) if g == 1 else None
            pass1_group(0, g, box, extra=extra)
        emit_gram(0, NG - 2)
        emit_gram(0, NG - 1)
        ab0 = stats(0)

        # pass 1 of sample 1, with sample 0's pass 2 interleaved; the last
        # 8 units are held back and woven into stats(1)'s DVE chains so the
        # PE never idles long enough for HAM to re-throttle before pass 2
        psgs[1] = psgp.tile([9, 4, 9], F32, tag="gram", name="psg1")
        box = [None]
        for g in range(NG):
            def extra(g=g):
                if g >= 4:
                    return
                for ot in range(4):
                    pass2_unit(0, g, ot, g * 4 + ot, ab0)
            pass1_group(1, g, box, extra=extra)
        emit_gram(1, NG - 2)
        emit_gram(1, NG - 1)
        # held-back sample-0 units fill the PE while the stats DVE chains
        # run; their stg copies go to ACT so they don't delay those chains
        wred1 = stats_a(1)
        for ot in range(4):
            pass2_pair(0, 4, ot, 16 + 2 * ot, ab0, stg_eng="scalar")
        murow1 = stats_b(1, wred1)
        for ot in range(4):
            pass2_pair(0, 6, ot, 24 + 2 * ot, ab0, stg_eng="scalar")
        ab1 = stats_c(1, murow1)

        # pass 2 of sample 1: paired units, psum tiles rotate three rings
        for g0 in range(0, NG, 2):
            for ot in range(4):
                pass2_pair(1, g0, ot, g0 * 4 + 2 * ot, ab1)

    nc.finalize()
    return nc


_NC_CACHE = {}


def _get_nc():
    if "nc" not in _NC_CACHE:
        _NC_CACHE["nc"] = build_nc()
    return _NC_CACHE["nc"]


def _fold_host(inputs):
    """Host-side weight folding + staging (shared across cores)."""
    k_w = np.asarray(inputs["k_w"], np.float32)
    v_w = np.asarray(inputs["v_w"], np.float32)
    to_q_w = np.asarray(inputs["to_q_w"], np.float32)
    to_k_w = np.asarray(inputs["to_k_w"], np.float32)
    to_v_w = np.asarray(inputs["to_v_w"], np.float32)
    out_w = np.asarray(inputs["out_w"], np.float32)
    out_b = np.asarray(inputs["out_b"], np.float32)
    gn_g = np.asarray(inputs["gn_g"], np.float32)
    gn_b = np.asarray(inputs["gn_b"], np.float32)

    qwT = np.ascontiguousarray(
        to_q_w.T.reshape(4, 128, C).transpose(1, 0, 2)
    ).astype(BF)  # [128, ct, he]
    kT = k_w.T.reshape(6, 128, C).transpose(1, 0, 2)  # [128, dt, o]
    vT = v_w.T.reshape(6, 128, C).transpose(1, 0, 2)
    kvT = np.ascontiguousarray(np.concatenate([kT, vT], axis=2)).astype(BF)

    rs_k = to_k_w.sum(axis=1)  # [C]
    rs_v = to_v_w.sum(axis=1)
    rskb = np.ascontiguousarray(np.broadcast_to(rs_k[None, :], (128, C))).astype(
        np.float32
    )

    # W2[o, h] = scale * sum_e out_w[o, h*64+e] * rs_v[h*64+e]
    W2 = SCALE * np.einsum(
        "ohe,he->oh", out_w.reshape(C, HEADS, DHEAD), rs_v.reshape(HEADS, DHEAD)
    )  # [C, HEADS]
    w2T = np.ascontiguousarray(
        W2.reshape(4, 128, HEADS).transpose(2, 0, 1)
    ).astype(BF)  # [h, ot, p]

    # Gm/Cm: sum mm^q = sum_ab M[a,b] S2[a,b], S2 = [s;1][s;1]^T over n
    G = W2.T @ W2  # [8, 8]
    colsumW2 = W2.sum(axis=0)  # [8]
    bW2 = out_b @ W2  # [8]
    Gm = np.zeros((9, 9), np.float32)
    Gm[:8, :8] = G
    Gm[8, :8] = bW2
    Gm[:8, 8] = bW2
    Gm[8, 8] = float((out_b ** 2).sum())
    Cm = np.zeros((9, 9), np.float32)
    Cm[8, :8] = colsumW2 / 2.0
    Cm[:8, 8] = colsumW2 / 2.0
    Cm[8, 8] = float(out_b.sum())
    gcm = np.ascontiguousarray(
        np.stack([Cm, Gm], axis=1)
    ).astype(np.float32)  # [9, 2, 9]

    omask = np.zeros((128, 4, HEADS), np.float32)
    for ht in range(4):
        for p in range(128):
            omask[p, ht, 2 * ht + p // 64] = 1.0
    omask = omask.astype(BF)

    cols = np.ascontiguousarray(
        np.stack(
            [*gn_g.reshape(4, 128), *gn_b.reshape(4, 128), *out_b.reshape(4, 128)],
            axis=1,
        )
    ).astype(np.float32)  # [128, 12]
    return dict(qwT=qwT, kvT=kvT, rskb=rskb, omask=omask, w2T=w2T, gcm=gcm, cols=cols)


def make_in_maps(inputs):
    x = np.asarray(inputs["x"], np.float32).reshape(B, 4, 128, N)
    x = np.ascontiguousarray(x).astype(BF)
    y = np.asarray(inputs["y"], np.float32).reshape(B, DIMY)
    shared = _fold_host(inputs)
    in_maps = []
    for core in range(NCORES):
        s0 = core * BPC
        yc = y[s0 : s0 + BPC]  # [BPC, DIMY]
        yT = np.ascontiguousarray(
            yc.T.reshape(6, 128, BPC).transpose(1, 0, 2)
        ).astype(BF)
        m = {"x": x[s0 : s0 + BPC].transpose(0, 2, 1, 3).copy(), "yT": yT}
        m.update(shared)
        in_maps.append(m)
    return in_maps


def kernel(**inputs):
    nc = _get_nc()
    res = run_bass_kernel_spmd(nc, make_in_maps(inputs), list(range(NCORES)))
    out = np.concatenate([r["out"] for r in res.results], axis=0)  # [B, 4, 128, N] bf16
    return out.astype(np.float32).reshape(B, C, 64, 64)


if __name__ == "__main__":
    rng = np.random.default_rng(0)
    inputs = {
        "x": rng.standard_normal((B, C, 64, 64), dtype=np.float32),
        "y": rng.standard_normal((B, 1, 1, DIMY), dtype=np.float32),
        "k_w": rng.standard_normal((C, DIMY), dtype=np.float32) * 0.02,
        "v_w": rng.standard_normal((C, DIMY), dtype=np.float32) * 0.02,
        "to_q_w": rng.standard_normal((C, C), dtype=np.float32) * 0.02,
        "to_k_w": rng.standard_normal((C, C), dtype=np.float32) * 0.02,
        "to_v_w": rng.standard_normal((C, C), dtype=np.float32) * 0.02,
        "out_w": rng.standard_normal((C, C), dtype=np.float32) * 0.02,
        "out_b": np.zeros(C, np.float32),
        "gn_g": np.ones(C, np.float32),
        "gn_b": np.zeros(C, np.float32),
    }
    out = kernel(**inputs)
    print("kernel ran, out shape", out.shape, "std", out.std())
